# revision 2
# baseline (speedup 1.0000x reference)
"""MinGRU forward on 8 Trainium2 NeuronCores.

Reference computation (per batch b):
    k       = x @ Wz + bz                 # [T, H]
    z       = sigmoid(k)
    c       = 1 - z
    htilde  = g(x @ Wh + bh)              # g(a) = a+0.5 if a>=0 else sigmoid(a)
                                          #      = max(a+0.5, sigmoid(a))
    h[0]    = g(h_0)
    h[t]    = c[t-1]*h[t-1] + z[t-1]*htilde[t-1]   (t = 1..T)
    out     = h                           # [T+1, H]

The log-space cumlogsumexp in the reference is exactly this linear
recurrence (all quantities positive, coefficients in (0,1), so the
linear form is numerically stable).

Sharding: data-parallel over batch, one batch per core, weights
replicated.

The kernel is Tensor-engine bound (1024 fp16 matmuls/core = 218.5us at
2.4GHz; fp8 DoubleRow was measured at ~1 cycle/row so error-compensated
fp8 is slower than fp16). v4 therefore optimizes PE occupancy at the
edges:
  - x is transposed AND cast to fp16 on the host, so the device issues
    only plain contiguous DMAs. The baseline's device-side DMA-transpose
    serialized the weight loads behind it (the first weight DMA waited
    on the transpose semaphore), costing ~8us of PE idle at kernel
    start.
  - x chunk DMAs ride the (otherwise idle) GpSimd DGE ring, weights the
    sync ring: both streams start immediately and in parallel. Chunk 0
    is loaded k-slice by k-slice so the first matmul only waits for
    128KB + the first weight slice.
  - Gates run fp16 end to end (z, s, c, g, v, h): DVE gets 2x
    throughput on 16-bit SBUF operands, the output DMA halves, and ACT
    drops from 3 sigmoids to 2 (c = 1-z moves to a cheap DVE
    tensor_scalar). GpSimd is not used at all (its software multiply is
    ~4x slower than DVE fp16 and adds drain overhead at the tail).
  - The scan keeps fp32 state internally (hardware guarantee) and only
    stores h as fp16; rel err stays ~2.4e-3 (limit 2e-2).
  - The last 512 timesteps are processed as two 256 chunks so the final
    gate/scan chain after the last matmul is short.
The device writes the output transposed ([H, T+1] fp16); the host
transposes and upcasts during the unshard.
"""

import numpy as np

B, T, D, H = 8, 4096, 1024, 1024
P = 128
TCH = 512                 # time-chunk (one PSUM bank of fp32 per matmul)
KO = D // P               # contraction tiles
MO = H // P               # output-channel tiles
# 7 full chunks + 2 half chunks at the end to shorten the tail
CHUNKS = [(i * TCH, TCH) for i in range(7)] + [(3584, 256), (3840, 256)]
NTCH = T // TCH           # host x layout is uniform 512-chunk-major

_PROGRAM_CACHE = {}


def _build_program():
    import concourse.bacc as bacc
    import concourse.mybir as mybir
    import concourse.tile as tile

    fp32 = mybir.dt.float32
    fp16 = mybir.dt.float16
    SIG = mybir.ActivationFunctionType.Sigmoid
    MUL = mybir.AluOpType.mult
    ADD = mybir.AluOpType.add
    MAX = mybir.AluOpType.max

    nc = bacc.Bacc("TRN2", target_bir_lowering=False)

    # x pre-transposed on host: [ki, nt, ko, t] with D-index = ko*128+ki,
    # T-index = nt*512+t  (chunk-major so each chunk DMA reads 8KB runs)
    xt_ext = nc.declare_dram_parameter("xt", [P, NTCH * KO * TCH], fp16, isOutput=False)
    h0_ext = nc.declare_dram_parameter("h_0", [H], fp32, isOutput=False)
    wz_ext = nc.declare_dram_parameter("Wz", [D, H], fp16, isOutput=False)
    bz_ext = nc.declare_dram_parameter("bz", [H], fp32, isOutput=False)
    wh_ext = nc.declare_dram_parameter("Wh", [D, H], fp16, isOutput=False)
    bh_ext = nc.declare_dram_parameter("bh", [H], fp32, isOutput=False)
    # transposed fp16 output; host untransposes/upcasts during the gather
    out_ext = nc.declare_dram_parameter("out", [H, T + 1], fp16, isOutput=True)

    xt_r = xt_ext.rearrange("p (nt ko t) -> p nt ko t", nt=NTCH, ko=KO)

    with tile.TileContext(nc) as tc:
        with (
            tc.tile_pool(name="const", bufs=1) as const_pool,
            tc.tile_pool(name="w", bufs=1) as w_pool,
            tc.tile_pool(name="xt", bufs=3) as xt_pool,
            tc.tile_pool(name="ht", bufs=2) as ht_pool,
            tc.tile_pool(name="gate", bufs=3) as gate_pool,
            tc.tile_pool(name="psp", bufs=4, space="PSUM") as psum_p,
        ):
            # Chunk 0's x, k-slice by k-slice on the GpSimd DGE ring (no
            # other traffic), so the first matmul waits only for slice 0.
            xt_first = xt_pool.tile([P, KO, TCH], fp16, tag="xt512", name="xt512")
            for ko in range(KO):
                nc.gpsimd.dma_start(xt_first[:, ko], xt_r[:, 0, ko])

            # Weights resident: [ki, ko, h] so lhsT tiles are natural slices.
            # Loaded per k-slice (contiguous 256KB each) on the sync ring, in
            # the order chunk 0's k-outer matmul schedule consumes them.
            wz_sb = w_pool.tile([P, KO, H], fp16)
            wh_sb = w_pool.tile([P, KO, H], fp16)
            wz_r = wz_ext.rearrange("(ko ki) h -> ki ko h", ki=P)
            wh_r = wh_ext.rearrange("(ko ki) h -> ki ko h", ki=P)
            for ko in range(KO):
                nc.sync.dma_start(wz_sb[:, ko], wz_r[:, ko])
                nc.sync.dma_start(wh_sb[:, ko], wh_r[:, ko])

            # Per-channel columns: partition = channel-within-tile, free = tile.
            bz_sb = const_pool.tile([P, MO], fp32)
            nc.sync.dma_start(bz_sb, bz_ext.rearrange("(mo mi) -> mi mo", mi=P))
            bh_sb = const_pool.tile([P, MO], fp32)
            nc.sync.dma_start(bh_sb, bh_ext.rearrange("(mo mi) -> mi mo", mi=P))
            bhp5_sb = const_pool.tile([P, MO], fp32)
            nc.vector.tensor_scalar_add(bhp5_sb, bh_sb, 0.5)

            # h[0] = g(h_0) = max(h_0 + 0.5, sigmoid(h_0)); fp32 for the scan
            # init plus an fp16 copy for the out[:, 0] write.
            h0_sb = const_pool.tile([P, MO], fp32)
            nc.sync.dma_start(h0_sb, h0_ext.rearrange("(mo mi) -> mi mo", mi=P))
            s0_sb = const_pool.tile([P, MO], fp32)
            nc.scalar.activation(s0_sb, h0_sb, SIG)
            gh0_sb = const_pool.tile([P, MO], fp32)
            nc.vector.scalar_tensor_tensor(gh0_sb, h0_sb, 0.5, s0_sb, op0=ADD, op1=MAX)
            gh0_16 = const_pool.tile([P, MO], fp16)
            nc.vector.tensor_scalar_mul(gh0_16, gh0_sb, 1.0)
            nc.sync.dma_start(out_ext[:, 0].rearrange("(mo mi) -> mi mo", mi=P), gh0_16)

            # x chunks prefetched two ahead on the GpSimd ring. Tiles are
            # tagged by width so the 256-wide tail chunks rotate separately.
            xt_tiles = [xt_first]

            def issue_xt(ci):
                t0, tch = CHUNKS[ci]
                xt_sb = xt_pool.tile(
                    [P, KO, tch], fp16, tag=f"xt{tch}", name=f"xt{tch}"
                )
                nt, toff = divmod(t0, TCH)
                nc.gpsimd.dma_start(xt_sb, xt_r[:, nt, :, toff:toff + tch])
                xt_tiles.append(xt_sb)

            issue_xt(1)

            prev_ht = None  # previous chunk's scan output (carries the state)
            prev_tch = TCH

            def gates_scan_store(m, t0, tch, pk, pa, ht_sb):
                # ACT order s -> z: s feeds the longest chain (g -> v -> scan)
                s_sb = gate_pool.tile([P, TCH], fp16, tag="s", name="s")[:, :tch]
                nc.scalar.activation(s_sb, pa, SIG, bias=bh_sb[:, m:m + 1])
                z_sb = gate_pool.tile([P, TCH], fp16, tag="z", name="z")[:, :tch]
                nc.scalar.activation(z_sb, pk, SIG, bias=bz_sb[:, m:m + 1])
                # g = max(a + bh + 0.5, s) straight from PSUM
                g_sb = gate_pool.tile([P, TCH], fp16, tag="g", name="g")[:, :tch]
                nc.vector.scalar_tensor_tensor(
                    g_sb, pa, bhp5_sb[:, m:m + 1], s_sb, op0=ADD, op1=MAX
                )
                c_sb = gate_pool.tile([P, TCH], fp16, tag="c", name="c")[:, :tch]
                nc.vector.tensor_scalar(c_sb, z_sb, -1.0, 1.0, op0=MUL, op1=ADD)
                v_sb = gate_pool.tile([P, TCH], fp16, tag="v", name="v")[:, :tch]
                nc.vector.tensor_mul(v_sb, z_sb, g_sb)

                init = (
                    gh0_sb[:, m:m + 1]
                    if prev_ht is None
                    else prev_ht[:, m, prev_tch - 1:prev_tch]
                )
                nc.vector.tensor_tensor_scan(
                    ht_sb[:, m, :tch], c_sb, v_sb, init, op0=MUL, op1=ADD
                )
                nc.sync.dma_start(
                    out_ext[m * P:(m + 1) * P, 1 + t0:1 + t0 + tch],
                    ht_sb[:, m, :tch],
                )

            for ci, (t0, tch) in enumerate(CHUNKS):
                if ci + 2 < len(CHUNKS):
                    issue_xt(ci + 2)
                xt_sb = xt_tiles[ci]
                ht_sb = ht_pool.tile([P, MO, TCH], fp16)

                if ci == 0:
                    # k-outer over groups of 4 output tiles: matmuls consume
                    # the weight k-slices in DMA arrival order, so the PE
                    # starts ~15us earlier and never stalls on weight loads
                    # (which would also re-throttle the HAM clock gate).
                    GQ = min(4, MO)
                    for half in range(MO // GQ):
                        pks = [
                            psum_p.tile([P, TCH], fp32, tag="pk", name="pk")
                            for _ in range(GQ)
                        ]
                        pas = [
                            psum_p.tile([P, TCH], fp32, tag="pa", name="pa")
                            for _ in range(GQ)
                        ]
                        for ko in range(KO):
                            for q in range(GQ):
                                m = half * GQ + q
                                nc.tensor.matmul(
                                    pks[q],
                                    wz_sb[:, ko, m * P:(m + 1) * P],
                                    xt_sb[:, ko, :],
                                    start=(ko == 0),
                                    stop=(ko == KO - 1),
                                )
                            for q in range(GQ):
                                m = half * GQ + q
                                nc.tensor.matmul(
                                    pas[q],
                                    wh_sb[:, ko, m * P:(m + 1) * P],
                                    xt_sb[:, ko, :],
                                    start=(ko == 0),
                                    stop=(ko == KO - 1),
                                )
                        for q in range(GQ):
                            m = half * GQ + q
                            gates_scan_store(m, t0, tch, pks[q], pas[q], ht_sb)
                else:
                    for m in range(MO):
                        pk = psum_p.tile([P, TCH], fp32, tag="pk", name="pk")[:, :tch]
                        pa = psum_p.tile([P, TCH], fp32, tag="pa", name="pa")[:, :tch]
                        for ko in range(KO):
                            nc.tensor.matmul(
                                pk,
                                wz_sb[:, ko, m * P:(m + 1) * P],
                                xt_sb[:, ko, :tch],
                                start=(ko == 0),
                                stop=(ko == KO - 1),
                            )
                        for ko in range(KO):
                            nc.tensor.matmul(
                                pa,
                                wh_sb[:, ko, m * P:(m + 1) * P],
                                xt_sb[:, ko, :tch],
                                start=(ko == 0),
                                stop=(ko == KO - 1),
                            )
                        gates_scan_store(m, t0, tch, pk, pa, ht_sb)

                prev_ht = ht_sb
                prev_tch = tch

    nc.finalize()
    return nc


def _get_program():
    if "v4" not in _PROGRAM_CACHE:
        _PROGRAM_CACHE["v4"] = _build_program()
    return _PROGRAM_CACHE["v4"]


def _prep_xt(xb):
    # [T, D] fp32 -> fp16 [ki, nt, ko, tch] with D = ko*128+ki, T = nt*512+tch
    x16 = np.asarray(xb, dtype=np.float16)
    xt = x16.reshape(NTCH, TCH, KO, P).transpose(3, 0, 2, 1)
    return np.ascontiguousarray(xt).reshape(P, NTCH * KO * TCH)


def run(x, h_0, Wz, bz, Wh, bh, trace=False):
    from concourse.bass_utils import run_bass_kernel_spmd

    nc = _get_program()
    wz16 = np.ascontiguousarray(np.asarray(Wz, dtype=np.float16))
    wh16 = np.ascontiguousarray(np.asarray(Wh, dtype=np.float16))
    bz32 = np.ascontiguousarray(np.asarray(bz, dtype=np.float32))
    bh32 = np.ascontiguousarray(np.asarray(bh, dtype=np.float32))
    in_maps = [
        {
            "xt": _prep_xt(x[b]),
            "h_0": np.ascontiguousarray(
                np.asarray(h_0[b], dtype=np.float32).reshape(H)
            ),
            "Wz": wz16,
            "bz": bz32,
            "Wh": wh16,
            "bh": bh32,
        }
        for b in range(B)
    ]
    res = run_bass_kernel_spmd(nc, in_maps, list(range(B)), trace=trace)
    out = np.stack(
        [res.results[b]["out"].T.astype(np.float32) for b in range(B)], axis=0
    )
    return np.ascontiguousarray(out), res


def kernel(x, h_0, Wz, bz, Wh, bh):
    out, _ = run(x, h_0, Wz, bz, Wh, bh)
    return out


# revision 7
# speedup vs baseline: 1.0254x; 1.0254x over previous
"""MinGRU forward on 8 Trainium2 NeuronCores.

Reference computation (per batch b):
    k       = x @ Wz + bz                 # [T, H]
    z       = sigmoid(k)
    c       = 1 - z
    htilde  = g(x @ Wh + bh)              # g(a) = a+0.5 if a>=0 else sigmoid(a)
                                          #      = max(a+0.5, sigmoid(a))
    h[0]    = g(h_0)
    h[t]    = c[t-1]*h[t-1] + z[t-1]*htilde[t-1]   (t = 1..T)
    out     = h                           # [T+1, H]

The log-space cumlogsumexp in the reference is exactly this linear
recurrence (all quantities positive, coefficients in (0,1), so the
linear form is numerically stable).

Sharding: data-parallel over batch, one batch per core, weights
replicated.

The kernel is Tensor-engine bound (1024 fp16 matmuls/core = 218.5us at
2.4GHz; fp8 DoubleRow was measured at ~1 cycle/row so error-compensated
fp8 is slower than fp16). v4 therefore optimizes PE occupancy at the
edges:
  - x is transposed AND cast to fp16 on the host, so the device issues
    only plain contiguous DMAs. The baseline's device-side DMA-transpose
    serialized the weight loads behind it (the first weight DMA waited
    on the transpose semaphore), costing ~8us of PE idle at kernel
    start.
  - x chunk DMAs ride the (otherwise idle) GpSimd DGE ring, weights the
    sync ring: both streams start immediately and in parallel. Chunk 0
    is loaded k-slice by k-slice so the first matmul only waits for
    128KB + the first weight slice.
  - Gates run fp16 end to end (z, s, c, g, v, h): DVE gets 2x
    throughput on 16-bit SBUF operands, the output DMA halves, and ACT
    drops from 3 sigmoids to 2 (c = 1-z moves to a cheap DVE
    tensor_scalar). GpSimd is not used at all (its software multiply is
    ~4x slower than DVE fp16 and adds drain overhead at the tail).
  - The scan keeps fp32 state internally (hardware guarantee) and only
    stores h as fp16; rel err stays ~2.4e-3 (limit 2e-2).
  - The last 512 timesteps are processed as two 256 chunks so the final
    gate/scan chain after the last matmul is short.
The device writes the output transposed ([H, T+1] fp16); the host
transposes and upcasts during the unshard.
"""

import numpy as np

B, T, D, H = 8, 4096, 1024, 1024
P = 128
TCH = 512                 # time-chunk (one PSUM bank of fp32 per matmul)
KO = D // P               # contraction tiles
MO = H // P               # output-channel tiles
# 7 full chunks + 2 half chunks at the end to shorten the tail
CHUNKS = [(i * TCH, TCH) for i in range(7)] + [(3584, 256), (3840, 256)]
NTCH = T // TCH           # host x layout is uniform 512-chunk-major

_PROGRAM_CACHE = {}


def _build_program():
    import concourse.bacc as bacc
    import concourse.mybir as mybir
    import concourse.tile as tile

    fp32 = mybir.dt.float32
    fp16 = mybir.dt.float16
    SIG = mybir.ActivationFunctionType.Sigmoid
    MUL = mybir.AluOpType.mult
    ADD = mybir.AluOpType.add
    MAX = mybir.AluOpType.max

    nc = bacc.Bacc("TRN2", target_bir_lowering=False)

    # x pre-transposed on host: [ki, nt, ko, t] with D-index = ko*128+ki,
    # T-index = nt*512+t  (chunk-major so each chunk DMA reads 8KB runs)
    xt_ext = nc.declare_dram_parameter("xt", [P, NTCH * KO * TCH], fp16, isOutput=False)
    h0_ext = nc.declare_dram_parameter("h_0", [H], fp32, isOutput=False)
    wz_ext = nc.declare_dram_parameter("Wz", [D, H], fp16, isOutput=False)
    bz_ext = nc.declare_dram_parameter("bz", [H], fp32, isOutput=False)
    wh_ext = nc.declare_dram_parameter("Wh", [D, H], fp16, isOutput=False)
    bh_ext = nc.declare_dram_parameter("bh", [H], fp32, isOutput=False)
    # transposed fp16 output; host untransposes/upcasts during the gather
    out_ext = nc.declare_dram_parameter("out", [H, T + 1], fp16, isOutput=True)

    xt_r = xt_ext.rearrange("p (nt ko t) -> p nt ko t", nt=NTCH, ko=KO)

    with tile.TileContext(nc) as tc:
        with (
            tc.tile_pool(name="const", bufs=1) as const_pool,
            tc.tile_pool(name="w", bufs=1) as w_pool,
            tc.tile_pool(name="xt", bufs=3) as xt_pool,
            tc.tile_pool(name="ht", bufs=2) as ht_pool,
            tc.tile_pool(name="gate", bufs=3) as gate_pool,
            tc.tile_pool(name="psp", bufs=4, space="PSUM") as psum_p,
        ):
            # Chunk 0's x, k-slice by k-slice on the GpSimd DGE ring (no
            # other traffic), so the first matmul waits only for slice 0.
            xt_first = xt_pool.tile([P, KO, TCH], fp16, tag="xt512", name="xt512")
            for ko in range(KO):
                nc.gpsimd.dma_start(xt_first[:, ko], xt_r[:, 0, ko])

            # Weights resident: [ki, ko, h] so lhsT tiles are natural slices.
            # Loaded per k-slice (contiguous 256KB each) on the sync ring, in
            # the order chunk 0's k-outer matmul schedule consumes them
            # (pa/wh first). Later x chunks also ride the sync ring BEHIND
            # the weights: the ring is in-order, so the weight stream gets
            # the DMA bandwidth until it is done (chunk 1 is not needed for
            # ~28us; letting it race the weights stalled the PE ~8us).
            wz_sb = w_pool.tile([P, KO, H], fp16)
            wh_sb = w_pool.tile([P, KO, H], fp16)
            wz_r = wz_ext.rearrange("(ko ki) h -> ki ko h", ki=P)
            wh_r = wh_ext.rearrange("(ko ki) h -> ki ko h", ki=P)
            for ko in range(KO):
                nc.sync.dma_start(wh_sb[:, ko], wh_r[:, ko])
                nc.sync.dma_start(wz_sb[:, ko], wz_r[:, ko])

            # Small constants ride the (otherwise unused) ACT DGE ring so
            # they arrive early without delaying the weight stream.
            bz_sb = const_pool.tile([P, MO], fp32)
            nc.scalar.dma_start(bz_sb, bz_ext.rearrange("(mo mi) -> mi mo", mi=P))
            bh_sb = const_pool.tile([P, MO], fp32)
            nc.scalar.dma_start(bh_sb, bh_ext.rearrange("(mo mi) -> mi mo", mi=P))
            bhp5_sb = const_pool.tile([P, MO], fp32)
            nc.vector.tensor_scalar_add(bhp5_sb, bh_sb, 0.5)

            # h[0] = g(h_0) = max(h_0 + 0.5, sigmoid(h_0)); fp32 for the scan
            # init plus an fp16 copy for the out[:, 0] write.
            h0_sb = const_pool.tile([P, MO], fp32)
            nc.scalar.dma_start(h0_sb, h0_ext.rearrange("(mo mi) -> mi mo", mi=P))
            s0_sb = const_pool.tile([P, MO], fp32)
            nc.scalar.activation(s0_sb, h0_sb, SIG)
            gh0_sb = const_pool.tile([P, MO], fp32)
            nc.vector.scalar_tensor_tensor(gh0_sb, h0_sb, 0.5, s0_sb, op0=ADD, op1=MAX)
            gh0_16 = const_pool.tile([P, MO], fp16)
            nc.vector.tensor_scalar_mul(gh0_16, gh0_sb, 1.0)
            nc.sync.dma_start(out_ext[:, 0].rearrange("(mo mi) -> mi mo", mi=P), gh0_16)

            # x chunks prefetched two ahead on the GpSimd ring. Tiles are
            # tagged by width so the 256-wide tail chunks rotate separately.
            xt_tiles = [xt_first]

            def issue_xt(ci):
                t0, tch = CHUNKS[ci]
                xt_sb = xt_pool.tile(
                    [P, KO, tch], fp16, tag=f"xt{tch}", name=f"xt{tch}"
                )
                nt, toff = divmod(t0, TCH)
                nc.sync.dma_start(xt_sb, xt_r[:, nt, :, toff:toff + tch])
                xt_tiles.append(xt_sb)

            issue_xt(1)

            prev_ht = None  # previous chunk's scan output (carries the state)
            prev_tch = TCH

            def gates_scan_store(m, t0, tch, pk, pa, ht_sb):
                # ACT order s -> z: s feeds the longest chain (g -> v -> scan)
                s_sb = gate_pool.tile([P, TCH], fp16, tag="s", name="s")[:, :tch]
                nc.scalar.activation(s_sb, pa, SIG, bias=bh_sb[:, m:m + 1])
                z_sb = gate_pool.tile([P, TCH], fp16, tag="z", name="z")[:, :tch]
                nc.scalar.activation(z_sb, pk, SIG, bias=bz_sb[:, m:m + 1])
                # g = max(a + bh + 0.5, s) straight from PSUM
                g_sb = gate_pool.tile([P, TCH], fp16, tag="g", name="g")[:, :tch]
                nc.vector.scalar_tensor_tensor(
                    g_sb, pa, bhp5_sb[:, m:m + 1], s_sb, op0=ADD, op1=MAX
                )
                c_sb = gate_pool.tile([P, TCH], fp16, tag="c", name="c")[:, :tch]
                nc.vector.tensor_scalar(c_sb, z_sb, -1.0, 1.0, op0=MUL, op1=ADD)
                v_sb = gate_pool.tile([P, TCH], fp16, tag="v", name="v")[:, :tch]
                nc.vector.tensor_mul(v_sb, z_sb, g_sb)

                init = (
                    gh0_sb[:, m:m + 1]
                    if prev_ht is None
                    else prev_ht[:, m, prev_tch - 1:prev_tch]
                )
                nc.vector.tensor_tensor_scan(
                    ht_sb[:, m, :tch], c_sb, v_sb, init, op0=MUL, op1=ADD
                )
                nc.sync.dma_start(
                    out_ext[m * P:(m + 1) * P, 1 + t0:1 + t0 + tch],
                    ht_sb[:, m, :tch],
                )

            for ci, (t0, tch) in enumerate(CHUNKS):
                if ci + 2 < len(CHUNKS):
                    issue_xt(ci + 2)
                xt_sb = xt_tiles[ci]
                ht_sb = ht_pool.tile([P, MO, TCH], fp16)

                if ci == 0:
                    # k-outer over groups of 4 output tiles: matmuls consume
                    # the weight k-slices in DMA arrival order, so the PE
                    # starts ~15us earlier and never stalls on weight loads
                    # (which would also re-throttle the HAM clock gate).
                    GQ = min(4, MO)
                    for half in range(MO // GQ):
                        pks = [
                            psum_p.tile([P, TCH], fp32, tag="pk", name="pk")
                            for _ in range(GQ)
                        ]
                        pas = [
                            psum_p.tile([P, TCH], fp32, tag="pa", name="pa")
                            for _ in range(GQ)
                        ]
                        # pa first: s/g/v feed the longest downstream chain
                        for ko in range(KO):
                            for q in range(GQ):
                                m = half * GQ + q
                                nc.tensor.matmul(
                                    pas[q],
                                    wh_sb[:, ko, m * P:(m + 1) * P],
                                    xt_sb[:, ko, :],
                                    start=(ko == 0),
                                    stop=(ko == KO - 1),
                                )
                            for q in range(GQ):
                                m = half * GQ + q
                                nc.tensor.matmul(
                                    pks[q],
                                    wz_sb[:, ko, m * P:(m + 1) * P],
                                    xt_sb[:, ko, :],
                                    start=(ko == 0),
                                    stop=(ko == KO - 1),
                                )
                        for q in range(GQ):
                            m = half * GQ + q
                            gates_scan_store(m, t0, tch, pks[q], pas[q], ht_sb)
                else:
                    for m in range(MO):
                        pk = psum_p.tile([P, TCH], fp32, tag="pk", name="pk")[:, :tch]
                        pa = psum_p.tile([P, TCH], fp32, tag="pa", name="pa")[:, :tch]
                        for ko in range(KO):
                            nc.tensor.matmul(
                                pa,
                                wh_sb[:, ko, m * P:(m + 1) * P],
                                xt_sb[:, ko, :tch],
                                start=(ko == 0),
                                stop=(ko == KO - 1),
                            )
                        for ko in range(KO):
                            nc.tensor.matmul(
                                pk,
                                wz_sb[:, ko, m * P:(m + 1) * P],
                                xt_sb[:, ko, :tch],
                                start=(ko == 0),
                                stop=(ko == KO - 1),
                            )
                        gates_scan_store(m, t0, tch, pk, pa, ht_sb)

                prev_ht = ht_sb
                prev_tch = tch

    nc.finalize()
    return nc


def _get_program():
    if "v4" not in _PROGRAM_CACHE:
        _PROGRAM_CACHE["v4"] = _build_program()
    return _PROGRAM_CACHE["v4"]


def _prep_xt(xb):
    # [T, D] fp32 -> fp16 [ki, nt, ko, tch] with D = ko*128+ki, T = nt*512+tch
    x16 = np.asarray(xb, dtype=np.float16)
    xt = x16.reshape(NTCH, TCH, KO, P).transpose(3, 0, 2, 1)
    return np.ascontiguousarray(xt).reshape(P, NTCH * KO * TCH)


def run(x, h_0, Wz, bz, Wh, bh, trace=False):
    from concourse.bass_utils import run_bass_kernel_spmd

    nc = _get_program()
    wz16 = np.ascontiguousarray(np.asarray(Wz, dtype=np.float16))
    wh16 = np.ascontiguousarray(np.asarray(Wh, dtype=np.float16))
    bz32 = np.ascontiguousarray(np.asarray(bz, dtype=np.float32))
    bh32 = np.ascontiguousarray(np.asarray(bh, dtype=np.float32))
    in_maps = [
        {
            "xt": _prep_xt(x[b]),
            "h_0": np.ascontiguousarray(
                np.asarray(h_0[b], dtype=np.float32).reshape(H)
            ),
            "Wz": wz16,
            "bz": bz32,
            "Wh": wh16,
            "bh": bh32,
        }
        for b in range(B)
    ]
    res = run_bass_kernel_spmd(nc, in_maps, list(range(B)), trace=trace)
    out = np.stack(
        [res.results[b]["out"].T.astype(np.float32) for b in range(B)], axis=0
    )
    return np.ascontiguousarray(out), res


def kernel(x, h_0, Wz, bz, Wh, bh):
    out, _ = run(x, h_0, Wz, bz, Wh, bh)
    return out
